# revision 34
# baseline (speedup 1.0000x reference)
"""Causal single-head attention (B=4, S=2048, D=1024) on 8 TRN2 NeuronCores.

Sharding: 2 cores per batch; each core owns 8 q-blocks of 128 rows chosen so
both cores of a batch see the same multiset of causal kv-span lengths:
core h=0 -> q-blocks [0,3,4,7,8,11,12,15], core h=1 -> [1,2,5,6,9,10,13,14];
padded pair-spans W = 256*(pos+1). One SPMD program serves all 8 cores;
per-core differences (which q rows, causal mask offsets) ride in the data.

Math per core (bf16 operands, fp32 PSUM accumulation), with the host folding
M = Wq @ Wk^T / sqrt(D) so no K-projection is needed on device:
  A^T = M^T @ qT                                      (single projection)
  S_i = A_i^T.T @ kT (+ additive causal mask)         (scores vs RAW k^T)
  P = exp(S), denom = rowsum(P)                       (no max-sub: |S| small)
  U_i = P @ v                                         (unnormalized, raw v)
  out_i = (U_i @ Wv) * (1/denom)                      (scale folded into the
                                                       out-proj PSUM copy)

v3 structure (vs v1):
  - A-proj is k-OUTER (4 passes of 4 concurrent PSUM groups), fed by small
    per-ko DMA granules, so the first matmul issues ~1.5us after the
    preamble instead of waiting for the full 3MB of wq+qT.
  - P^T still uses PE transposes but they ride a slot-stamped pending
    queue (transpose at chunk-slot+1, AV at +2, out-proj at fin+3) so the
    in-order PE queue never blocks on the mask+exp chain.
  - U^T (per block) uses one XBAR DMA-transpose on the sync queue; the
    1/denom scale is folded into the out-proj PSUM->SBUF copy, dropping
    normalize from the critical path. One merged out-DMA per block. Total
    DMA count stays ~v1-level: more DMAs thrash the framework's DMA
    completion-semaphore windows (measured 12-27us rotation stalls).
"""

import os
from collections import deque

import ml_dtypes
import numpy as np

import concourse.bass as bass
import concourse.mybir as mybir
import concourse.tile as tile
from concourse import bacc
from concourse.bass_utils import run_bass_kernel_spmd

B, S, D = 4, 2048, 1024
P = 128                      # partitions / q-block rows
NBLK = 8                     # q-blocks per core
CH = 512                     # kv chunk (matmul moving free dim)
KO = D // P                  # 8 contraction chunks
NV = S // P                  # 16 v row-chunks
MW = 256                     # mask window (diagonal 128 + max pad 128)
W = [256, 512, 768, 1024, 1280, 1536, 1792, 2048]   # padded pair spans
BLOCKS = [[0, 3, 4, 7, 8, 11, 12, 15], [1, 2, 5, 6, 9, 10, 13, 14]]
ORDER = list(range(NBLK))   # ascending spans: big last block hides finishes
BF = mybir.dt.bfloat16
F8 = mybir.dt.float8e4
F32 = mybir.dt.float32
NEG = -1e30
NSPIN = 22
T_TAGS = ["T0e", "T1e", "T0o", "T1o"]
ASCALE = 64.0               # fp8 scores: A carries x64 (folded into wq on host)

_cached = {}


def _build():
    if "nc" in _cached:
        return _cached["nc"]
    nc = bacc.Bacc("TRN2", target_bir_lowering=False, debug=False, num_devices=8)
    qT = nc.dram_tensor("qT", [D, P * NBLK], BF, kind="ExternalInput").ap()
    kT = nc.dram_tensor("kT", [D, S], F8, kind="ExternalInput").ap()
    v = nc.dram_tensor("v", [S, D], BF, kind="ExternalInput").ap()
    wq = nc.dram_tensor("wq", [D, D], BF, kind="ExternalInput").ap()
    wv = nc.dram_tensor("wv", [D, D], BF, kind="ExternalInput").ap()
    mask = nc.dram_tensor("mask", [P, NBLK, MW], BF, kind="ExternalInput").ap()
    out = nc.dram_tensor("out", [P * NBLK, D], BF, kind="ExternalOutput").ap()

    kT_r = kT.rearrange("(kk two p) s -> p kk two s", p=P, two=2)
    v_r = v.rearrange("(so p) d -> p so d", p=P)
    wv_r = wv.rearrange("(ko p) m -> p ko m", p=P)
    wq_r = wq.rearrange("(ko p) m -> p ko m", p=P)
    qT_r = qT.rearrange("(ko p) s -> p ko s", p=P)

    Act = mybir.ActivationFunctionType

    with tile.TileContext(nc) as tc:
        with tc.tile_pool(name="pers", bufs=1) as pers, \
             tc.tile_pool(name="qw", bufs=1) as qw, \
             tc.tile_pool(name="ppool", bufs=4) as ppool, \
             tc.tile_pool(name="ptpool", bufs=4) as ptpool, \
             tc.tile_pool(name="tpool", bufs=2) as tpool, \
             tc.tile_pool(name="ttpool", bufs=2) as ttpool, \
             tc.tile_pool(name="opool", bufs=4) as opool, \
             tc.tile_pool(name="cwork", bufs=2) as cwork, \
             tc.tile_pool(name="pswork", bufs=2, space="PSUM") as pswork, \
             tc.tile_pool(name="ps_s", bufs=2, space="PSUM") as ps_s, \
             tc.tile_pool(name="ps_t", bufs=1, space="PSUM") as ps_t:

            # identity built on-device (memset + affine diag select): no DMA
            ones_sb = pers.tile([P, P], BF)
            ident_sb = pers.tile([P, P], BF)
            nc.vector.memset(ones_sb[:], 1.0)
            nc.gpsimd.affine_select(ident_sb[:], ones_sb[:],
                                    pattern=[[-1, P]],
                                    compare_op=mybir.AluOpType.is_equal,
                                    fill=0.0, base=0, channel_multiplier=1)
            # preload the scalar-engine Exp table before the hot loop
            warm_in = pers.tile([P, 1], F32)
            nc.vector.memset(warm_in[:], 0.0)
            warm_out = pers.tile([P, 1], BF)
            nc.scalar.activation(warm_out[:], warm_in[:], Act.Exp)

            mask_sb = pers.tile([P, NBLK, MW], BF)
            QT_sb = pers.tile([P, KO // 2, 2, P * NBLK], F8)
            KT_sb = pers.tile([P, KO // 2, 2, S], F8)
            V_sb = pers.tile([P, NV, D], BF)
            WV_sb = pers.tile([P, KO, D], BF)
            qT_sb = qw.tile([P, KO, P * NBLK], BF)
            wq_sb = qw.tile([P, KO, D], BF)

            # ---- DMA emission (sync queue), first-use order. A-proj is
            # k-outer, so per-ko granules unblock compute early; each
            # descriptor costs ~600ns to issue, so later granules batch up.
            nc.sync.dma_start(wq_sb[:, 0:1, 0:CH], wq_r[:, 0:1, 0:CH])
            nc.sync.dma_start(qT_sb[:, 0:1, 0:CH], qT_r[:, 0:1, 0:CH])
            nc.sync.dma_start(wq_sb[:, 1:2, 0:CH], wq_r[:, 1:2, 0:CH])
            nc.sync.dma_start(qT_sb[:, 1:2, 0:CH], qT_r[:, 1:2, 0:CH])
            for g in range(2, KO, 2):
                nc.sync.dma_start(wq_sb[:, g:g + 2, 0:CH],
                                  wq_r[:, g:g + 2, 0:CH])
                nc.sync.dma_start(qT_sb[:, g:g + 2, 0:CH],
                                  qT_r[:, g:g + 2, 0:CH])
            for g in range(0, KO, 2):
                nc.sync.dma_start(wq_sb[:, g:g + 2, CH:D],
                                  wq_r[:, g:g + 2, CH:D])
            for g in range(0, KO, 2):
                nc.sync.dma_start(qT_sb[:, g:g + 2, CH:D],
                                  qT_r[:, g:g + 2, CH:D])
            nc.sync.dma_start(KT_sb[:, :, :, 0:S // 2], kT_r[:, :, :, 0:S // 2])
            nc.sync.dma_start(KT_sb[:, :, :, S // 2:S], kT_r[:, :, :, S // 2:S])
            nc.sync.dma_start(mask_sb[:], mask)
            nc.sync.dma_start(V_sb[:, 0:4], v_r[:, 0:4])
            nc.sync.dma_start(V_sb[:, 4:8], v_r[:, 4:8])
            nc.sync.dma_start(V_sb[:, 8:NV], v_r[:, 8:NV])
            nc.sync.dma_start(WV_sb[:], wv_r[:])

            # Warm the PE clock-gate while the first granules stream.
            spin_ps = ps_s.tile([P, CH], F32, tag="s", name="spin_ps")
            for si in range(NSPIN):
                nc.tensor.matmul(spin_ps[:, 0:P], ones_sb[:], ones_sb[:],
                                 start=(si == 0), stop=(si == NSPIN - 1))
            spin_out = cwork.tile([P, P], BF, tag="spin", name="spin_out")
            nc.vector.tensor_copy(spin_out[:], spin_ps[:, 0:P])

            # ---- A-projection: 4 passes (n-half x m-half), k outermost so
            # each pass streams 4 concurrent PSUM groups fed by per-ko DMAs.
            for pa in range(4):
                n, mh = pa // 2, pa % 2
                tiles = [ps_t.tile([P, CH], F32, tag=T_TAGS[mi],
                                   name=f"apj_{pa}_{mi}") for mi in range(4)]
                for k in range(KO):
                    for mi in range(4):
                        m = mh * 4 + mi
                        nc.tensor.matmul(
                            tiles[mi][:], wq_sb[:, k, bass.ts(m, P)],
                            qT_sb[:, k, bass.ts(n, CH)],
                            start=(k == 0), stop=(k == KO - 1))
                for mi in range(4):
                    m = mh * 4 + mi
                    nc.vector.tensor_copy(
                        QT_sb[:, m // 2, m % 2, bass.ts(n, CH)], tiles[mi][:])

            # ---- attention pipeline ----
            # Jobs are stamped with the earliest scores-chunk slot they may
            # be emitted at, so AV trails its chunk by 2 slots (covers the
            # exp -> XBAR-transpose-DMA latency) and each out-proj trails
            # its U-transpose-DMA by 2 slots. FIFO order is preserved.
            pending = deque()
            slot_box = [0]

            def pump_ready():
                while pending and pending[0][0] <= slot_box[0]:
                    pending.popleft()[1]()

            def chunk_widths(wi):
                nfull, rem = divmod(wi, CH)
                return [CH] * nfull + ([rem] if rem else [])

            def emit_scores(pos, c, w, st):
                last = (c * CH + w == W[pos])
                ps_c = ps_s.tile([P, CH], F32, tag="s", name=f"s_{pos}_{c}")
                for kk in range(KO // 2):
                    nc.tensor.matmul(
                        ps_c[:, 0:w], QT_sb[:, kk, :, bass.ts(pos, P)],
                        KT_sb[:, kk, :, bass.ds(c * CH, w)],
                        start=(kk == 0), stop=(kk == KO // 2 - 1),
                        perf_mode=mybir.MatmulPerfMode.DoubleRow)
                if last:
                    nc.vector.tensor_tensor(
                        ps_c[:, w - MW:w], ps_c[:, w - MW:w],
                        mask_sb[:, pos, :], mybir.AluOpType.add)
                if w == CH:
                    p_sb = ppool.tile([P, CH], BF, tag="p", name=f"p_{pos}_{c}")
                    pt_sb = ptpool.tile([P, CH // P, P], BF, tag="pt",
                                        name=f"pt_{pos}_{c}")
                else:
                    p_sb = ppool.tile([P, w], BF, tag="p2", bufs=2,
                                      name=f"p_{pos}_{c}")
                    pt_sb = ptpool.tile([P, w // P, P], BF, tag="pt2", bufs=2,
                                        name=f"pt_{pos}_{c}")
                ds_t = cwork.tile([P, 1], F32, tag="ds", bufs=8,
                                  name=f"ds_{pos}_{c}")
                nc.scalar.activation(p_sb[:], ps_c[:, 0:w], Act.Exp,
                                     scale=1.0 / ASCALE, accum_out=ds_t[:])
                st["dsums"].append(ds_t)
                return p_sb, pt_sb

            def ptr_job(pos, c, w, p_sb, pt_sb):
                def run():
                    for t in range(w // P):
                        ptr = pswork.tile([P, P], BF, tag="tr",
                                          name=f"ptr_{pos}_{c}_{t}")
                        nc.tensor.transpose(
                            ptr[:], p_sb[:, bass.ts(t, P)], ident_sb[:])
                        if t % 2 == 0:
                            nc.vector.tensor_copy(pt_sb[:, t, :], ptr[:])
                        else:
                            nc.scalar.activation(pt_sb[:, t, :], ptr[:],
                                                 Act.Copy)
                return run

            def av_job(pos, c, w, pt_sb, st):
                def run():
                    nkv = W[pos] // P
                    for t in range(w // P):
                        kvi = c * (CH // P) + t
                        vc = V_sb[:, kvi]
                        nc.tensor.matmul(
                            st["T0"][:], pt_sb[:, t, :], vc[:, 0:CH],
                            start=(kvi == 0), stop=(kvi == nkv - 1))
                        nc.tensor.matmul(
                            st["T1"][:], pt_sb[:, t, :], vc[:, CH:D],
                            start=(kvi == 0), stop=(kvi == nkv - 1))
                return run

            def fin1_job(pos, st):
                def run():
                    dsums = st["dsums"]
                    den = cwork.tile([P, 1], F32, tag="den", name=f"den_{pos}")
                    if len(dsums) == 1:
                        nc.vector.tensor_copy(den[:], dsums[0][:])
                    else:
                        nc.vector.tensor_tensor(den[:], dsums[0][:],
                                                dsums[1][:],
                                                mybir.AluOpType.add)
                        for dsx in dsums[2:]:
                            nc.vector.tensor_tensor(den[:], den[:], dsx[:],
                                                    mybir.AluOpType.add)
                    rden = cwork.tile([P, 1], F32, tag="rden",
                                      name=f"rden_{pos}")
                    nc.vector.reciprocal(rden[:], den[:])
                    st["rden"] = rden
                    t_sb = tpool.tile([P, D], BF, tag="t", name=f"t_{pos}")
                    tt_sb = ttpool.tile([P, KO, P], BF, tag="tt",
                                        name=f"tt_{pos}")
                    if st["tail"]:
                        # final block: XBAR-DMA latency (~3us) would sit on
                        # the critical tail; PE transposes start right off
                        # the U-copy instead.
                        for half in range(2):
                            nc.vector.tensor_copy(
                                t_sb[:, bass.ts(half, CH)],
                                st["T" + str(half)][:])
                            for dc in range(4 * half, 4 * half + 4):
                                ptr = pswork.tile([P, P], BF, tag="tr",
                                                  name=f"ttr_{pos}_{dc}")
                                nc.tensor.transpose(
                                    ptr[:], t_sb[:, bass.ts(dc, P)],
                                    ident_sb[:])
                                nc.vector.tensor_copy(tt_sb[:, dc, :], ptr[:])
                    else:
                        nc.vector.tensor_copy(t_sb[:, 0:CH], st["T0"][:])
                        nc.vector.tensor_copy(t_sb[:, CH:D], st["T1"][:])
                        nc.sync.dma_start_transpose(tt_sb[:], t_sb[:])
                    st["tt"] = tt_sb
                return run

            def fin2_job(pos, st, par):
                def run():
                    # out-proj halves reuse this block's own AV banks: free
                    # since fin1's U-copy, next needed by AV two blocks on.
                    tt_sb = st["tt"]
                    o_sb = opool.tile([P, D], BF, tag="o", name=f"o_{pos}")
                    for half in range(2):
                        ps_o = ps_t.tile([P, CH], F32, tag=f"T{half}{par}",
                                         name=f"o{half}_{pos}")
                        for dc in range(KO):
                            nc.tensor.matmul(
                                ps_o[:], tt_sb[:, dc, :],
                                WV_sb[:, dc, bass.ts(half, CH)],
                                start=(dc == 0), stop=(dc == KO - 1))
                        nc.scalar.activation(
                            o_sb[:, bass.ts(half, CH)], ps_o[:],
                            Act.Copy, scale=st["rden"][:])
                    nc.sync.dma_start(out[bass.ts(pos, P), :], o_sb[:])
                return run

            for idx, pos in enumerate(ORDER):
                par = "e" if idx % 2 == 0 else "o"
                st = {
                    "dsums": [],
                    "tail": idx == NBLK - 1,
                    "T0": ps_t.tile([P, CH], F32, tag=f"T0{par}",
                                    name=f"T0_{pos}"),
                    "T1": ps_t.tile([P, CH], F32, tag=f"T1{par}",
                                    name=f"T1_{pos}"),
                }
                for c, w in enumerate(chunk_widths(W[pos])):
                    p_sb, pt_sb = emit_scores(pos, c, w, st)
                    pending.append((slot_box[0] + 1,
                                    ptr_job(pos, c, w, p_sb, pt_sb)))
                    pending.append((slot_box[0] + 2,
                                    av_job(pos, c, w, pt_sb, st)))
                    slot_box[0] += 1
                    pump_ready()
                pending.append((slot_box[0] + 1, fin1_job(pos, st)))
                pending.append((slot_box[0] + 3, fin2_job(pos, st, par)))
            while pending:
                pending.popleft()[1]()

    nc.compile()
    _cached["nc"] = nc
    return nc


LAST_RESULT = None


def kernel(q, k, v, Wq, Wk, Wv, mask):
    global LAST_RESULT
    q = np.asarray(q, dtype=np.float32)
    k = np.asarray(k, dtype=np.float32)
    v = np.asarray(v, dtype=np.float32)
    Wq = np.asarray(Wq, dtype=np.float32)
    Wk = np.asarray(Wk, dtype=np.float32)
    Wv = np.asarray(Wv, dtype=np.float32)

    nc = _build()

    bf = ml_dtypes.bfloat16
    f8 = ml_dtypes.float8_e4m3
    wm = np.ascontiguousarray(
        (Wq.astype(np.float64) @ Wk.astype(np.float64).T
         / np.sqrt(np.float64(D)) * ASCALE).astype(bf))
    wv_c = np.ascontiguousarray(Wv.astype(bf))

    # additive causal masks for the last MW columns of each block's padded
    # span: column c' maps to global kv = (W-MW)+c', masked when kv > q0+r.
    masks = []
    r = np.arange(P)[:, None]
    c = np.arange(MW)[None, :]
    for h in range(2):
        m = np.zeros((P, NBLK, MW), dtype=np.float32)
        for i in range(NBLK):
            j = BLOCKS[h][i]
            q0 = P * j
            kv = (W[i] - MW) + c
            m[:, i, :] = np.where(kv <= q0 + r, 0.0, NEG)
        masks.append(m.astype(bf))

    in_maps = []
    for core in range(8):
        b, h = core // 2, core % 2
        blocks = BLOCKS[h]
        qTb = q[b].T  # [D, S]
        cols = np.concatenate([np.arange(j * P, (j + 1) * P) for j in blocks])
        in_maps.append({
            "qT": np.ascontiguousarray(qTb[:, cols].astype(bf)),
            "kT": np.ascontiguousarray(k[b].T.astype(f8)),
            "v": np.ascontiguousarray(v[b].astype(bf)),
            "wq": wm, "wv": wv_c,
            "mask": masks[h],
        })

    res = run_bass_kernel_spmd(nc, in_maps, list(range(8)),
                               trace=bool(os.environ.get("KERNEL_TRACE")))
    LAST_RESULT = res

    out = np.empty((B, S, D), dtype=np.float32)
    for core in range(8):
        b, h = core // 2, core % 2
        oc = np.asarray(res.results[core]["out"], dtype=np.float32)
        for pos, j in enumerate(BLOCKS[h]):
            out[b, j * P:(j + 1) * P, :] = oc[pos * P:(pos + 1) * P, :]
    return out


# revision 35
# speedup vs baseline: 1.1684x; 1.1684x over previous
"""Causal single-head attention (B=4, S=2048, D=1024) on 8 TRN2 NeuronCores.

Sharding: 2 cores per batch; each core owns 8 q-blocks of 128 rows chosen so
both cores of a batch see the same multiset of causal kv-span lengths:
core h=0 -> q-blocks [0,3,4,7,8,11,12,15], core h=1 -> [1,2,5,6,9,10,13,14];
padded pair-spans W = 256*(pos+1). One SPMD program serves all 8 cores;
per-core differences (which q rows, causal mask offsets) ride in the data.

Math per core (bf16 operands, fp32 PSUM accumulation), with the host folding
M = Wq @ Wk^T / sqrt(D) so no K-projection is needed on device:
  A^T = M^T @ qT                                      (single projection)
  S_i = A_i^T.T @ kT (+ additive causal mask)         (scores vs RAW k^T)
  P = exp(S), denom = rowsum(P)                       (no max-sub: |S| small)
  U_i = P @ v                                         (unnormalized, raw v)
  out_i = (U_i @ Wv) * (1/denom)                      (scale folded into the
                                                       out-proj PSUM copy)

v3 structure (vs v1):
  - A-proj is k-OUTER (4 passes of 4 concurrent PSUM groups), fed by small
    per-ko DMA granules, so the first matmul issues ~1.5us after the
    preamble instead of waiting for the full 3MB of wq+qT.
  - P^T still uses PE transposes but they ride a slot-stamped pending
    queue (transpose at chunk-slot+1, AV at +2, out-proj at fin+3) so the
    in-order PE queue never blocks on the mask+exp chain.
  - U^T (per block) uses one XBAR DMA-transpose on the sync queue; the
    1/denom scale is folded into the out-proj PSUM->SBUF copy, dropping
    normalize from the critical path. One merged out-DMA per block. Total
    DMA count stays ~v1-level: more DMAs thrash the framework's DMA
    completion-semaphore windows (measured 12-27us rotation stalls).
"""

import os
from collections import deque

import ml_dtypes
import numpy as np

import concourse.bass as bass
import concourse.mybir as mybir
import concourse.tile as tile
from concourse import bacc
from concourse.bass_utils import run_bass_kernel_spmd

B, S, D = 4, 2048, 1024
P = 128                      # partitions / q-block rows
NBLK = 8                     # q-blocks per core
CH = 512                     # kv chunk (matmul moving free dim)
KO = D // P                  # 8 contraction chunks
NV = S // P                  # 16 v row-chunks
MW = 256                     # mask window (diagonal 128 + max pad 128)
W = [256, 512, 768, 1024, 1280, 1536, 1792, 2048]   # padded pair spans
BLOCKS = [[0, 3, 4, 7, 8, 11, 12, 15], [1, 2, 5, 6, 9, 10, 13, 14]]
ORDER = list(range(NBLK))   # ascending spans: big last block hides finishes
BF = mybir.dt.bfloat16
F8 = mybir.dt.float8e4
F32 = mybir.dt.float32
NEG = -1e30
NSPIN = 22
T_TAGS = ["T0e", "T1e", "T0o", "T1o"]
ASCALE = 64.0               # fp8 scores: A carries x64 (folded into wq on host)

_cached = {}


def _build():
    if "nc" in _cached:
        return _cached["nc"]
    nc = bacc.Bacc("TRN2", target_bir_lowering=False, debug=False, num_devices=8)
    qT = nc.dram_tensor("qT", [D, P * NBLK], BF, kind="ExternalInput").ap()
    kT = nc.dram_tensor("kT", [D, S], F8, kind="ExternalInput").ap()
    v = nc.dram_tensor("v", [S, D], BF, kind="ExternalInput").ap()
    wq = nc.dram_tensor("wq", [D, D], BF, kind="ExternalInput").ap()
    wv = nc.dram_tensor("wv", [D, D], BF, kind="ExternalInput").ap()
    mask = nc.dram_tensor("mask", [P, NBLK, MW], BF, kind="ExternalInput").ap()
    out = nc.dram_tensor("out", [P * NBLK, D], BF, kind="ExternalOutput").ap()

    kT_r = kT.rearrange("(kk two p) s -> p kk two s", p=P, two=2)
    v_r = v.rearrange("(so p) d -> p so d", p=P)
    wv_r = wv.rearrange("(ko p) m -> p ko m", p=P)
    wq_r = wq.rearrange("(ko p) m -> p ko m", p=P)
    qT_r = qT.rearrange("(ko p) s -> p ko s", p=P)

    Act = mybir.ActivationFunctionType

    with tile.TileContext(nc) as tc:
        with tc.tile_pool(name="pers", bufs=1) as pers, \
             tc.tile_pool(name="qw", bufs=1) as qw, \
             tc.tile_pool(name="ppool", bufs=4) as ppool, \
             tc.tile_pool(name="ptpool", bufs=4) as ptpool, \
             tc.tile_pool(name="tpool", bufs=2) as tpool, \
             tc.tile_pool(name="ttpool", bufs=2) as ttpool, \
             tc.tile_pool(name="opool", bufs=4) as opool, \
             tc.tile_pool(name="cwork", bufs=2) as cwork, \
             tc.tile_pool(name="pswork", bufs=2, space="PSUM") as pswork, \
             tc.tile_pool(name="ps_s", bufs=2, space="PSUM") as ps_s, \
             tc.tile_pool(name="ps_t", bufs=1, space="PSUM") as ps_t:

            # identity built on-device (memset + affine diag select): no DMA
            ones_sb = pers.tile([P, P], BF)
            ident_sb = pers.tile([P, P], BF)
            nc.vector.memset(ones_sb[:], 1.0)
            nc.gpsimd.affine_select(ident_sb[:], ones_sb[:],
                                    pattern=[[-1, P]],
                                    compare_op=mybir.AluOpType.is_equal,
                                    fill=0.0, base=0, channel_multiplier=1)
            # preload the scalar-engine Exp table before the hot loop
            warm_in = pers.tile([P, 1], F32)
            nc.vector.memset(warm_in[:], 0.0)
            warm_out = pers.tile([P, 1], BF)
            nc.scalar.activation(warm_out[:], warm_in[:], Act.Exp)

            mask_sb = pers.tile([P, NBLK, MW], BF)
            QT_sb = pers.tile([P, KO // 2, 2, P * NBLK], F8)
            KT_sb = pers.tile([P, KO // 2, 2, S], F8)
            V_sb = pers.tile([P, NV, D], BF)
            WV_sb = pers.tile([P, KO, D], BF)
            qT_sb = qw.tile([P, KO, P * NBLK], BF)
            wq_sb = qw.tile([P, KO, D], BF)

            # ---- DMA emission (sync queue), first-use order. A-proj is
            # k-outer, so per-ko granules unblock compute early; each
            # descriptor costs ~600ns to issue, so later granules batch up.
            nc.sync.dma_start(wq_sb[:, 0:1, 0:CH], wq_r[:, 0:1, 0:CH])
            nc.sync.dma_start(qT_sb[:, 0:1, 0:CH], qT_r[:, 0:1, 0:CH])
            nc.sync.dma_start(wq_sb[:, 1:2, 0:CH], wq_r[:, 1:2, 0:CH])
            nc.sync.dma_start(qT_sb[:, 1:2, 0:CH], qT_r[:, 1:2, 0:CH])
            for g in range(2, KO, 2):
                nc.sync.dma_start(wq_sb[:, g:g + 2, 0:CH],
                                  wq_r[:, g:g + 2, 0:CH])
                nc.sync.dma_start(qT_sb[:, g:g + 2, 0:CH],
                                  qT_r[:, g:g + 2, 0:CH])
            for g in range(0, KO, 2):
                nc.sync.dma_start(wq_sb[:, g:g + 2, CH:D],
                                  wq_r[:, g:g + 2, CH:D])
            for g in range(0, KO, 2):
                nc.sync.dma_start(qT_sb[:, g:g + 2, CH:D],
                                  qT_r[:, g:g + 2, CH:D])
            nc.sync.dma_start(KT_sb[:, :, :, 0:S // 2], kT_r[:, :, :, 0:S // 2])
            nc.sync.dma_start(KT_sb[:, :, :, S // 2:S], kT_r[:, :, :, S // 2:S])
            nc.sync.dma_start(mask_sb[:], mask)
            nc.sync.dma_start(V_sb[:, 0:4], v_r[:, 0:4])
            nc.sync.dma_start(V_sb[:, 4:8], v_r[:, 4:8])
            nc.sync.dma_start(V_sb[:, 8:NV], v_r[:, 8:NV])
            nc.sync.dma_start(WV_sb[:], wv_r[:])

            # Warm the PE clock-gate while the first granules stream.
            spin_ps = ps_s.tile([P, CH], F32, tag="s", name="spin_ps")
            for si in range(NSPIN):
                nc.tensor.matmul(spin_ps[:, 0:P], ones_sb[:], ones_sb[:],
                                 start=(si == 0), stop=(si == NSPIN - 1))
            spin_out = cwork.tile([P, P], BF, tag="spin", name="spin_out")
            nc.vector.tensor_copy(spin_out[:], spin_ps[:, 0:P])

            # ---- A-projection: 4 passes (n-half x m-half), k outermost so
            # each pass streams 4 concurrent PSUM groups fed by per-ko DMAs.
            for pa in range(4):
                n, mh = pa // 2, pa % 2
                tiles = [ps_t.tile([P, CH], F32, tag=T_TAGS[mi],
                                   name=f"apj_{pa}_{mi}") for mi in range(4)]
                for k in range(KO):
                    for mi in range(4):
                        m = mh * 4 + mi
                        nc.tensor.matmul(
                            tiles[mi][:], wq_sb[:, k, bass.ts(m, P)],
                            qT_sb[:, k, bass.ts(n, CH)],
                            start=(k == 0), stop=(k == KO - 1))
                for mi in range(4):
                    m = mh * 4 + mi
                    nc.vector.tensor_copy(
                        QT_sb[:, m // 2, m % 2, bass.ts(n, CH)], tiles[mi][:])

            # ---- attention pipeline ----
            # Jobs are stamped with the earliest scores-chunk slot they may
            # be emitted at, so AV trails its chunk by 2 slots (covers the
            # exp -> XBAR-transpose-DMA latency) and each out-proj trails
            # its U-transpose-DMA by 2 slots. FIFO order is preserved.
            pending = deque()
            slot_box = [0]

            def pump_ready():
                while pending and pending[0][0] <= slot_box[0]:
                    pending.popleft()[1]()

            def chunk_widths(wi):
                nfull, rem = divmod(wi, CH)
                return [CH] * nfull + ([rem] if rem else [])

            def emit_scores(pos, c, w, st):
                last = (c * CH + w == W[pos])
                ps_c = ps_s.tile([P, CH], F32, tag="s", name=f"s_{pos}_{c}")
                for kk in range(KO // 2):
                    nc.tensor.matmul(
                        ps_c[:, 0:w], QT_sb[:, kk, :, bass.ts(pos, P)],
                        KT_sb[:, kk, :, bass.ds(c * CH, w)],
                        start=(kk == 0), stop=(kk == KO // 2 - 1),
                        perf_mode=mybir.MatmulPerfMode.DoubleRow)
                if last:
                    nc.vector.tensor_tensor(
                        ps_c[:, w - MW:w], ps_c[:, w - MW:w],
                        mask_sb[:, pos, :], mybir.AluOpType.add)
                if w == CH:
                    p_sb = ppool.tile([P, CH], BF, tag="p", name=f"p_{pos}_{c}")
                    pt_sb = ptpool.tile([P, CH // P, P], BF, tag="pt",
                                        name=f"pt_{pos}_{c}")
                else:
                    p_sb = ppool.tile([P, w], BF, tag="p2", bufs=2,
                                      name=f"p_{pos}_{c}")
                    pt_sb = ptpool.tile([P, w // P, P], BF, tag="pt2", bufs=2,
                                        name=f"pt_{pos}_{c}")
                ds_t = cwork.tile([P, 1], F32, tag="ds", bufs=8,
                                  name=f"ds_{pos}_{c}")
                nc.scalar.activation(p_sb[:], ps_c[:, 0:w], Act.Exp,
                                     scale=1.0 / ASCALE, accum_out=ds_t[:])
                st["dsums"].append(ds_t)
                return p_sb, pt_sb

            def ptr_job(pos, c, w, p_sb, pt_sb):
                def run():
                    for t in range(w // P):
                        ptr = pswork.tile([P, P], BF, tag="tr",
                                          name=f"ptr_{pos}_{c}_{t}")
                        nc.tensor.transpose(
                            ptr[:], p_sb[:, bass.ts(t, P)], ident_sb[:])
                        nc.vector.tensor_copy(pt_sb[:, t, :], ptr[:])
                return run

            def av_job(pos, c, w, pt_sb, st):
                def run():
                    nkv = W[pos] // P
                    for t in range(w // P):
                        kvi = c * (CH // P) + t
                        vc = V_sb[:, kvi]
                        nc.tensor.matmul(
                            st["T0"][:], pt_sb[:, t, :], vc[:, 0:CH],
                            start=(kvi == 0), stop=(kvi == nkv - 1))
                        nc.tensor.matmul(
                            st["T1"][:], pt_sb[:, t, :], vc[:, CH:D],
                            start=(kvi == 0), stop=(kvi == nkv - 1))
                return run

            def fin1_job(pos, st):
                def run():
                    dsums = st["dsums"]
                    den = cwork.tile([P, 1], F32, tag="den", name=f"den_{pos}")
                    if len(dsums) == 1:
                        nc.vector.tensor_copy(den[:], dsums[0][:])
                    else:
                        nc.vector.tensor_tensor(den[:], dsums[0][:],
                                                dsums[1][:],
                                                mybir.AluOpType.add)
                        for dsx in dsums[2:]:
                            nc.vector.tensor_tensor(den[:], den[:], dsx[:],
                                                    mybir.AluOpType.add)
                    rden = cwork.tile([P, 1], F32, tag="rden",
                                      name=f"rden_{pos}")
                    nc.vector.reciprocal(rden[:], den[:])
                    st["rden"] = rden
                    t_sb = tpool.tile([P, D], BF, tag="t", name=f"t_{pos}")
                    tt_sb = ttpool.tile([P, KO, P], BF, tag="tt",
                                        name=f"tt_{pos}")
                    if st["tail"]:
                        # final block: XBAR-DMA latency (~3us) would sit on
                        # the critical tail; PE transposes start right off
                        # the U-copy instead.
                        for half in range(2):
                            nc.vector.tensor_copy(
                                t_sb[:, bass.ts(half, CH)],
                                st["T" + str(half)][:])
                            for dc in range(4 * half, 4 * half + 4):
                                ptr = pswork.tile([P, P], BF, tag="tr",
                                                  name=f"ttr_{pos}_{dc}")
                                nc.tensor.transpose(
                                    ptr[:], t_sb[:, bass.ts(dc, P)],
                                    ident_sb[:])
                                nc.vector.tensor_copy(tt_sb[:, dc, :], ptr[:])
                    else:
                        nc.vector.tensor_copy(t_sb[:, 0:CH], st["T0"][:])
                        nc.vector.tensor_copy(t_sb[:, CH:D], st["T1"][:])
                        nc.sync.dma_start_transpose(tt_sb[:], t_sb[:])
                    st["tt"] = tt_sb
                return run

            def fin2_job(pos, st, par):
                def run():
                    # out-proj halves reuse this block's own AV banks: free
                    # since fin1's U-copy, next needed by AV two blocks on.
                    tt_sb = st["tt"]
                    o_sb = opool.tile([P, D], BF, tag="o", name=f"o_{pos}")
                    for half in range(2):
                        ps_o = ps_t.tile([P, CH], F32, tag=f"T{half}{par}",
                                         name=f"o{half}_{pos}")
                        for dc in range(KO):
                            nc.tensor.matmul(
                                ps_o[:], tt_sb[:, dc, :],
                                WV_sb[:, dc, bass.ts(half, CH)],
                                start=(dc == 0), stop=(dc == KO - 1))
                        nc.scalar.activation(
                            o_sb[:, bass.ts(half, CH)], ps_o[:],
                            Act.Copy, scale=st["rden"][:])
                    nc.sync.dma_start(out[bass.ts(pos, P), :], o_sb[:])
                return run

            for idx, pos in enumerate(ORDER):
                par = "e" if idx % 2 == 0 else "o"
                st = {
                    "dsums": [],
                    "tail": idx == NBLK - 1,
                    "T0": ps_t.tile([P, CH], F32, tag=f"T0{par}",
                                    name=f"T0_{pos}"),
                    "T1": ps_t.tile([P, CH], F32, tag=f"T1{par}",
                                    name=f"T1_{pos}"),
                }
                for c, w in enumerate(chunk_widths(W[pos])):
                    p_sb, pt_sb = emit_scores(pos, c, w, st)
                    pending.append((slot_box[0] + 1,
                                    ptr_job(pos, c, w, p_sb, pt_sb)))
                    pending.append((slot_box[0] + 2,
                                    av_job(pos, c, w, pt_sb, st)))
                    slot_box[0] += 1
                    pump_ready()
                pending.append((slot_box[0] + 1, fin1_job(pos, st)))
                pending.append((slot_box[0] + 3, fin2_job(pos, st, par)))
            while pending:
                pending.popleft()[1]()

    nc.compile()
    _cached["nc"] = nc
    return nc


LAST_RESULT = None


def kernel(q, k, v, Wq, Wk, Wv, mask):
    global LAST_RESULT
    q = np.asarray(q, dtype=np.float32)
    k = np.asarray(k, dtype=np.float32)
    v = np.asarray(v, dtype=np.float32)
    Wq = np.asarray(Wq, dtype=np.float32)
    Wk = np.asarray(Wk, dtype=np.float32)
    Wv = np.asarray(Wv, dtype=np.float32)

    nc = _build()

    bf = ml_dtypes.bfloat16
    f8 = ml_dtypes.float8_e4m3
    wm = np.ascontiguousarray(
        (Wq.astype(np.float64) @ Wk.astype(np.float64).T
         / np.sqrt(np.float64(D)) * ASCALE).astype(bf))
    wv_c = np.ascontiguousarray(Wv.astype(bf))

    # additive causal masks for the last MW columns of each block's padded
    # span: column c' maps to global kv = (W-MW)+c', masked when kv > q0+r.
    masks = []
    r = np.arange(P)[:, None]
    c = np.arange(MW)[None, :]
    for h in range(2):
        m = np.zeros((P, NBLK, MW), dtype=np.float32)
        for i in range(NBLK):
            j = BLOCKS[h][i]
            q0 = P * j
            kv = (W[i] - MW) + c
            m[:, i, :] = np.where(kv <= q0 + r, 0.0, NEG)
        masks.append(m.astype(bf))

    in_maps = []
    for core in range(8):
        b, h = core // 2, core % 2
        blocks = BLOCKS[h]
        qTb = q[b].T  # [D, S]
        cols = np.concatenate([np.arange(j * P, (j + 1) * P) for j in blocks])
        in_maps.append({
            "qT": np.ascontiguousarray(qTb[:, cols].astype(bf)),
            "kT": np.ascontiguousarray(k[b].T.astype(f8)),
            "v": np.ascontiguousarray(v[b].astype(bf)),
            "wq": wm, "wv": wv_c,
            "mask": masks[h],
        })

    res = run_bass_kernel_spmd(nc, in_maps, list(range(8)),
                               trace=bool(os.environ.get("KERNEL_TRACE")))
    LAST_RESULT = res

    out = np.empty((B, S, D), dtype=np.float32)
    for core in range(8):
        b, h = core // 2, core % 2
        oc = np.asarray(res.results[core]["out"], dtype=np.float32)
        for pos, j in enumerate(BLOCKS[h]):
            out[b, j * P:(j + 1) * P, :] = oc[pos * P:(pos + 1) * P, :]
    return out
